# revision 38
# baseline (speedup 1.0000x reference)
"""Trainium2 kernel for nn_CNNEncoder: embed(1000,3) -> 4x conv1d(stride3) -> relu -> 50x50 linear.

Math: the four stride-3 convs + concat are one linear map C [50, 60] over the
flattened embedding signal e = emb[src].reshape(B, 60). So per row:
    out = relu(e @ C.T + cb) @ Wp.T + bp

Device layout (pure data parallel over 8 cores, 65536 rows/core):
  - features on partitions, rows on the free dim (PE contracts over partitions)
  - two 32768-row chunks packed block-diagonally: stage-1 lhsT is [121, 101]
    (60 signal partitions per chunk + shared ones-row for the bias; col 100
    forwards the ones-row so stage 2 gets its bias row for free), stage-2 lhsT
    is [101, 100].
  - per 512-col subtile: matmul -> relu (ACT) -> matmul -> copy to SBUF (DVE),
    DMA in/out in 4096-col super-tiles.

Host side does only data movement: the embedding gather (index lookup, no
arithmetic) and transposes for the on-device layout. All FLOPs run on device.
"""

import os
import numpy as np

try:
    import concourse.bass as bass
except ImportError:  # grading env may not have concourse on sys.path
    import sys

    sys.path.insert(0, "/opt/trn_rl_repo")
    import concourse.bass as bass

import concourse.mybir as mybir
import concourse.tile as tile
from concourse import bacc
from concourse.bass import ds, ts
from concourse import bass_utils
from concourse.bass_utils import run_bass_kernel_spmd

# spread HWDGE DMAs across all 16 SDMA engines (default leaves some idle)
_orig_run_command = bass_utils.run_command


_FLAG = "--min-num-dma-engines-for-dge=16"
_flag_ok = None


def _walrus_supports_flag(walrus):
    global _flag_ok
    if _flag_ok is None:
        try:
            import subprocess

            out = subprocess.run(
                [walrus, "--help"], capture_output=True, text=True, timeout=120
            )
            _flag_ok = "--min-num-dma-engines-for-dge" in (out.stdout + out.stderr)
        except Exception:
            _flag_ok = False
    return _flag_ok


def _patched_run_command(argv, **kwargs):
    if (
        argv
        and "walrus_driver" in str(argv[0])
        and "--pass" in argv
        and _walrus_supports_flag(str(argv[0]))
    ):
        argv = list(argv) + [_FLAG]
    return _orig_run_command(argv, **kwargs)


bass_utils.run_command = _patched_run_command

B = 524288
SEQ = 20
EMB = 3
L = SEQ * EMB  # 60
F = 50
NCORES = 8
RPC = B // NCORES  # 65536 rows per core
HALF = RPC // 2  # 32768 rows per packed chunk
NT = HALF  # free dim of the per-core device tensors

KP1 = 2 * L + 1  # 121: [chunkA 60 | chunkB 60 | ones]
MP1 = 2 * F + 1  # 101: [chunkA 50 | chunkB 50 | ones passthrough]
KP2 = MP1  # 101
MP2 = 2 * F  # 100

DMA_N = 4096
SUB = 512

F32 = mybir.dt.float32
F16 = mybir.dt.float16

CONV_SPECS = [(10, 14), (12, 13), (13, 12), (15, 11)]  # (pad, n_out)

LAST_RESULTS = None  # BassKernelResults of the most recent run (for profiling)

_NC_CACHE = {}


def _build_C(w1, b1, w2, b2, w3, b3, w4, b4):
    C = np.zeros((F, L), np.float64)
    cb = np.zeros(F, np.float64)
    f = 0
    for (w, b), (pad, nout) in zip(
        [(w1, b1), (w2, b2), (w3, b3), (w4, b4)], CONV_SPECS
    ):
        wk = np.asarray(w, np.float64)[0, 0]
        K = wk.shape[0]
        for j in range(nout):
            for k in range(K):
                i = 3 * j + k - pad
                if 0 <= i < L:
                    C[f, i] += wk[k]
            cb[f] = np.asarray(b, np.float64)[0]
            f += 1
    return C.astype(np.float32), cb.astype(np.float32)


def _build_nc():
    if "nc" in _NC_CACHE:
        return _NC_CACHE["nc"]

    nc = bacc.Bacc("TRN2", target_bir_lowering=False, debug=False, num_devices=NCORES)
    et = nc.dram_tensor("et", [KP1, NT], F16, kind="ExternalInput").ap()
    w1d = nc.dram_tensor("w1d", [KP1, MP1], F16, kind="ExternalInput").ap()
    w2d = nc.dram_tensor("w2d", [KP2, MP2], F16, kind="ExternalInput").ap()
    o = nc.dram_tensor("o", [MP2, NT], F16, kind="ExternalOutput").ap()

    # column schedule: small tiles at the edges for fast ramp/drain, big
    # DMAs in steady state to amortize descriptor generation
    col_tiles = [1024] * 2 + [2048] * 15
    assert sum(col_tiles) == NT

    with tile.TileContext(nc) as tc:
        with (
            tc.tile_pool(name="consts", bufs=1) as consts,
            tc.tile_pool(name="inp", bufs=6) as inp,
            tc.tile_pool(name="hbuf", bufs=4) as hbuf,
            tc.tile_pool(name="obuf", bufs=6) as obuf,
            tc.tile_pool(name="ps1", bufs=3, space="PSUM") as ps1,
            tc.tile_pool(name="ps2", bufs=3, space="PSUM") as ps2,
        ):
            w1t = consts.tile([KP1, MP1], F16)
            nc.sync.dma_start(w1t[:], w1d[:])
            w2t = consts.tile([KP2, MP2], F16)
            nc.sync.dma_start(w2t[:], w2d[:])

            col = 0
            for i, ncols in enumerate(col_tiles):
                x = inp.tile([KP1, ncols], F16, tag="x")
                nc.sync.dma_start(x[:], et[:, col : col + ncols])
                ot = obuf.tile([MP2, ncols], F16, tag="ot")
                for j in range(ncols // SUB):
                    p1 = ps1.tile([MP1, SUB], F32)
                    nc.tensor.matmul(
                        p1[:], w1t[:], x[:, ts(j, SUB)], start=True, stop=True
                    )
                    h = hbuf.tile([KP2, SUB], F16)
                    nc.scalar.activation(
                        h[:], p1[:], mybir.ActivationFunctionType.Relu
                    )
                    p2 = ps2.tile([MP2, SUB], F32)
                    nc.tensor.matmul(p2[:], w2t[:], h[:], start=True, stop=True)
                    nc.vector.tensor_copy(ot[:, ts(j, SUB)], p2[:])
                # stores via SWDGE: descriptors spray across all 16 SDMA
                # engines, while HWDGE loads are pinned to the 11 model rows.
                # 1024-col store chunks start draining after two copies
                # instead of waiting out the whole supertile
                sc = 0
                while sc < ncols:
                    w = min(1024, ncols - sc)
                    nc.gpsimd.dma_start(
                        o[:, col + sc : col + sc + w], ot[:, sc : sc + w]
                    )
                    sc += w
                col += ncols

    nc.compile()
    _NC_CACHE["nc"] = nc
    return nc


def kernel(**inputs):
    global LAST_RESULTS
    src = np.asarray(inputs["src"])
    emb = np.asarray(inputs["emb"], np.float32)
    Wp = np.asarray(inputs["Wp"], np.float32)
    bp = np.asarray(inputs["bp"], np.float32)
    C, cb = _build_C(
        inputs["w1"], inputs["b1"], inputs["w2"], inputs["b2"],
        inputs["w3"], inputs["b3"], inputs["w4"], inputs["b4"],
    )

    # stage-1 stationary [121, 101]
    L1 = np.zeros((KP1, MP1), np.float16)
    L1[0:L, 0:F] = C.T
    L1[L : 2 * L, F : 2 * F] = C.T
    L1[2 * L, 0:F] = cb
    L1[2 * L, F : 2 * F] = cb
    L1[2 * L, 2 * F] = 1.0  # forward the ones-row (relu(1) == 1)

    # stage-2 stationary [101, 100]
    L2 = np.zeros((KP2, MP2), np.float16)
    L2[0:F, 0:F] = Wp.T
    L2[F : 2 * F, F : 2 * F] = Wp.T
    L2[2 * F, 0:F] = bp
    L2[2 * F, F : 2 * F] = bp

    # host gather + per-core transposed layout [121, 32768]
    e = emb[src]  # [B, 20, 3]
    in_maps = []
    for c in range(NCORES):
        blk = e[c * RPC : (c + 1) * RPC].reshape(2, HALF, L)
        ET = np.empty((KP1, NT), np.float16)
        ET[0 : 2 * L] = np.transpose(blk, (0, 2, 1)).reshape(2 * L, HALF)
        ET[2 * L] = 1.0
        in_maps.append({"et": ET, "w1d": L1, "w2d": L2})

    nc = _build_nc()
    trace = bool(int(os.environ.get("KERNEL_TRACE", "0")))
    res = run_bass_kernel_spmd(
        nc, in_maps, core_ids=list(range(NCORES)), trace=trace
    )
    LAST_RESULTS = res

    out = np.empty((B, F), np.float32)
    for c in range(NCORES):
        oc = res.results[c]["o"].astype(np.float32)
        out[c * RPC : c * RPC + HALF] = oc[0:F].T
        out[c * RPC + HALF : (c + 1) * RPC] = oc[F : 2 * F].T
    return out


# revision 39
# speedup vs baseline: 1.0003x; 1.0003x over previous
"""Trainium2 kernel for nn_CNNEncoder: embed(1000,3) -> 4x conv1d(stride3) -> relu -> 50x50 linear.

Math: the four stride-3 convs + concat are one linear map C [50, 60] over the
flattened embedding signal e = emb[src].reshape(B, 60). So per row:
    out = relu(e @ C.T + cb) @ Wp.T + bp

Device layout (pure data parallel over 8 cores, 65536 rows/core):
  - features on partitions, rows on the free dim (PE contracts over partitions)
  - two 32768-row chunks packed block-diagonally: stage-1 lhsT is [121, 101]
    (60 signal partitions per chunk + shared ones-row for the bias; col 100
    forwards the ones-row so stage 2 gets its bias row for free), stage-2 lhsT
    is [101, 100].
  - per 512-col subtile: matmul -> relu (ACT) -> matmul -> copy to SBUF (DVE),
    DMA in/out in 4096-col super-tiles.

Host side does only data movement: the embedding gather (index lookup, no
arithmetic) and transposes for the on-device layout. All FLOPs run on device.
"""

import os
import numpy as np

try:
    import concourse.bass as bass
except ImportError:  # grading env may not have concourse on sys.path
    import sys

    sys.path.insert(0, "/opt/trn_rl_repo")
    import concourse.bass as bass

import concourse.mybir as mybir
import concourse.tile as tile
from concourse import bacc
from concourse.bass import ds, ts
from concourse import bass_utils
from concourse.bass_utils import run_bass_kernel_spmd

# spread HWDGE DMAs across all 16 SDMA engines (default leaves some idle)
_orig_run_command = bass_utils.run_command


_FLAG = "--min-num-dma-engines-for-dge=16"
_flag_ok = None


def _walrus_supports_flag(walrus):
    global _flag_ok
    if _flag_ok is None:
        try:
            import subprocess

            out = subprocess.run(
                [walrus, "--help"], capture_output=True, text=True, timeout=120
            )
            _flag_ok = "--min-num-dma-engines-for-dge" in (out.stdout + out.stderr)
        except Exception:
            _flag_ok = False
    return _flag_ok


def _patched_run_command(argv, **kwargs):
    if (
        argv
        and "walrus_driver" in str(argv[0])
        and "--pass" in argv
        and _walrus_supports_flag(str(argv[0]))
    ):
        argv = list(argv) + [_FLAG]
    return _orig_run_command(argv, **kwargs)


bass_utils.run_command = _patched_run_command

B = 524288
SEQ = 20
EMB = 3
L = SEQ * EMB  # 60
F = 50
NCORES = 8
RPC = B // NCORES  # 65536 rows per core
HALF = RPC // 2  # 32768 rows per packed chunk
NT = HALF  # free dim of the per-core device tensors

KP1 = 2 * L + 1  # 121: [chunkA 60 | chunkB 60 | ones]
MP1 = 2 * F + 1  # 101: [chunkA 50 | chunkB 50 | ones passthrough]
KP2 = MP1  # 101
MP2 = 2 * F  # 100

DMA_N = 4096
SUB = 512

F32 = mybir.dt.float32
F16 = mybir.dt.float16

CONV_SPECS = [(10, 14), (12, 13), (13, 12), (15, 11)]  # (pad, n_out)

LAST_RESULTS = None  # BassKernelResults of the most recent run (for profiling)

_NC_CACHE = {}


def _build_C(w1, b1, w2, b2, w3, b3, w4, b4):
    C = np.zeros((F, L), np.float64)
    cb = np.zeros(F, np.float64)
    f = 0
    for (w, b), (pad, nout) in zip(
        [(w1, b1), (w2, b2), (w3, b3), (w4, b4)], CONV_SPECS
    ):
        wk = np.asarray(w, np.float64)[0, 0]
        K = wk.shape[0]
        for j in range(nout):
            for k in range(K):
                i = 3 * j + k - pad
                if 0 <= i < L:
                    C[f, i] += wk[k]
            cb[f] = np.asarray(b, np.float64)[0]
            f += 1
    return C.astype(np.float32), cb.astype(np.float32)


def _build_nc():
    if "nc" in _NC_CACHE:
        return _NC_CACHE["nc"]

    nc = bacc.Bacc("TRN2", target_bir_lowering=False, debug=False, num_devices=NCORES)
    et = nc.dram_tensor("et", [KP1, NT], F16, kind="ExternalInput").ap()
    w1d = nc.dram_tensor("w1d", [KP1, MP1], F16, kind="ExternalInput").ap()
    w2d = nc.dram_tensor("w2d", [KP2, MP2], F16, kind="ExternalInput").ap()
    o = nc.dram_tensor("o", [MP2, NT], F16, kind="ExternalOutput").ap()

    # column schedule: small tiles at the edges for fast ramp/drain, big
    # DMAs in steady state to amortize descriptor generation
    col_tiles = [1024] * 2 + [2048] * 15
    assert sum(col_tiles) == NT

    with tile.TileContext(nc) as tc:
        with (
            tc.tile_pool(name="consts", bufs=1) as consts,
            tc.tile_pool(name="inp", bufs=6) as inp,
            tc.tile_pool(name="hbuf", bufs=4) as hbuf,
            tc.tile_pool(name="obuf", bufs=6) as obuf,
            tc.tile_pool(name="ps1", bufs=3, space="PSUM") as ps1,
            tc.tile_pool(name="ps2", bufs=3, space="PSUM") as ps2,
        ):
            w1t = consts.tile([KP1, MP1], F16)
            nc.sync.dma_start(w1t[:], w1d[:])
            w2t = consts.tile([KP2, MP2], F16)
            nc.sync.dma_start(w2t[:], w2d[:])

            col = 0
            for i, ncols in enumerate(col_tiles):
                x = inp.tile([KP1, ncols], F16, tag="x")
                nc.sync.dma_start(x[:], et[:, col : col + ncols])
                ot = obuf.tile([MP2, ncols], F16, tag="ot")
                for j in range(ncols // SUB):
                    p1 = ps1.tile([MP1, SUB], F32)
                    nc.tensor.matmul(
                        p1[:], w1t[:], x[:, ts(j, SUB)], start=True, stop=True
                    )
                    h = hbuf.tile([KP2, SUB], F16)
                    nc.scalar.activation(
                        h[:], p1[:], mybir.ActivationFunctionType.Relu
                    )
                    p2 = ps2.tile([MP2, SUB], F32)
                    nc.tensor.matmul(p2[:], w2t[:], h[:], start=True, stop=True)
                    nc.vector.tensor_copy(ot[:, ts(j, SUB)], p2[:])
                # stores via SWDGE: descriptors spray across all 16 SDMA
                # engines, while HWDGE loads are pinned to the 11 model rows
                nc.gpsimd.dma_start(o[:, col : col + ncols], ot[:])
                col += ncols

    nc.compile()
    _NC_CACHE["nc"] = nc
    return nc


def kernel(**inputs):
    global LAST_RESULTS
    src = np.asarray(inputs["src"])
    emb = np.asarray(inputs["emb"], np.float32)
    Wp = np.asarray(inputs["Wp"], np.float32)
    bp = np.asarray(inputs["bp"], np.float32)
    C, cb = _build_C(
        inputs["w1"], inputs["b1"], inputs["w2"], inputs["b2"],
        inputs["w3"], inputs["b3"], inputs["w4"], inputs["b4"],
    )

    # stage-1 stationary [121, 101]
    L1 = np.zeros((KP1, MP1), np.float16)
    L1[0:L, 0:F] = C.T
    L1[L : 2 * L, F : 2 * F] = C.T
    L1[2 * L, 0:F] = cb
    L1[2 * L, F : 2 * F] = cb
    L1[2 * L, 2 * F] = 1.0  # forward the ones-row (relu(1) == 1)

    # stage-2 stationary [101, 100]
    L2 = np.zeros((KP2, MP2), np.float16)
    L2[0:F, 0:F] = Wp.T
    L2[F : 2 * F, F : 2 * F] = Wp.T
    L2[2 * F, 0:F] = bp
    L2[2 * F, F : 2 * F] = bp

    # host gather + per-core transposed layout [121, 32768]
    e = emb[src]  # [B, 20, 3]
    in_maps = []
    for c in range(NCORES):
        blk = e[c * RPC : (c + 1) * RPC].reshape(2, HALF, L)
        ET = np.empty((KP1, NT), np.float16)
        ET[0 : 2 * L] = np.transpose(blk, (0, 2, 1)).reshape(2 * L, HALF)
        ET[2 * L] = 1.0
        in_maps.append({"et": ET, "w1d": L1, "w2d": L2})

    nc = _build_nc()
    trace = bool(int(os.environ.get("KERNEL_TRACE", "0")))
    res = run_bass_kernel_spmd(
        nc, in_maps, core_ids=list(range(NCORES)), trace=trace
    )
    LAST_RESULTS = res

    out = np.empty((B, F), np.float32)
    for c in range(NCORES):
        oc = res.results[c]["o"].astype(np.float32)
        out[c * RPC : c * RPC + HALF] = oc[0:F].T
        out[c * RPC + HALF : (c + 1) * RPC] = oc[F : 2 * F].T
    return out
